# revision 31
# baseline (speedup 1.0000x reference)
"""Trainium2 Bass kernel: 8-member EnsembleCritic MLP (min-Q over ensemble).

Sharding: expert-parallel — one ensemble member per NeuronCore (E=8 on 8
cores). x is replicated; per-member weights go to their core. Each core
computes qs[e] = MLP_e(x) for its member; the host stacks the per-core
outputs and takes the min over members (the gather step).

Per-core math (batch tiles of 128 rows, batch-major layout [batch=partition,
feature=free]):
  L1:  h1 = x @ W1 + b1           4x K=128 bf16 matmuls + one K=1 bias matmul
  LN1: bn_stats/bn_aggr on PSUM fp32, normalize+ReLU fused in one ScalarE
       activation pass (scale=rsqrt(var+eps), bias=-mu*scale, func=Relu)
  L2:  h2 = relu1 @ W2 + b2       relu1 transposed via DMA-xbar (bf16)
  LN2 + ReLU, then q = sum(relu2 * W3) + b3 via one tensor_tensor_reduce.

Matmul operands are bf16 (full-rate PE, xbar-transposable); all
accumulation/statistics are fp32.
"""

import sys

import numpy as np

if "/opt/trn_rl_repo" not in sys.path:
    sys.path.insert(0, "/opt/trn_rl_repo")

import ml_dtypes

E = 8
D_IN = 512
H = 1024
LN_EPS = 1e-5
P = 128
BF = ml_dtypes.bfloat16

_PROG_CACHE = {}
_last_results = None  # test harness introspection


def _build_program(B, fast_affine, for_hw=True):
    import concourse.bass as bass
    import concourse.tile as tile
    from concourse import mybir
    from concourse.bass import ts

    f32 = mybir.dt.float32
    bf16 = mybir.dt.bfloat16
    AF = mybir.ActivationFunctionType
    OP = mybir.AluOpType

    NT = B // P
    KC1 = D_IN // P  # 4 contraction chunks for layer 1
    KC2 = H // P     # 8 contraction chunks for layer 2

    nc = bass.Bass()
    # real (never-waited) semaphore for the wait-splitting post-pass nops
    _dummy_sem_cm = nc.semaphore("twait_dummy")
    dummy_sem = _dummy_sem_cm.__enter__()
    x_d = nc.dram_tensor("x", [B, D_IN], bf16, kind="ExternalInput")
    w1_d = nc.dram_tensor("w1", [D_IN, H], bf16, kind="ExternalInput")
    w2_d = nc.dram_tensor("w2", [H, H], bf16, kind="ExternalInput")
    w3_d = nc.dram_tensor("w3", [1, H], bf16, kind="ExternalInput")
    b1_d = nc.dram_tensor("b1", [1, H], bf16, kind="ExternalInput")
    b2_d = nc.dram_tensor("b2", [1, H], bf16, kind="ExternalInput")
    b3_d = nc.dram_tensor("b3", [1, 1], f32, kind="ExternalInput")
    if not fast_affine:
        g1_d = nc.dram_tensor("g1", [1, H], bf16, kind="ExternalInput")
        be1_d = nc.dram_tensor("be1", [1, H], bf16, kind="ExternalInput")
        g2_d = nc.dram_tensor("g2", [1, H], bf16, kind="ExternalInput")
        be2_d = nc.dram_tensor("be2", [1, H], bf16, kind="ExternalInput")
    # q[p, t] holds the Q value of batch row t*128+p; host transposes.
    q_d = nc.dram_tensor("q", [P, NT], f32, kind="ExternalOutput")

    with tile.TileContext(nc) as tc:
        with (
            tc.tile_pool(name="weights", bufs=1) as wpool,
            tc.tile_pool(name="xt", bufs=3) as xtpool,
            tc.tile_pool(name="act", bufs=3) as apool,
            tc.tile_pool(name="rt", bufs=3) as rtpool,
            tc.tile_pool(name="stat", bufs=6) as stpool,
            tc.tile_pool(name="psum1", bufs=2, space="PSUM") as pp1,
            tc.tile_pool(name="psum2", bufs=2, space="PSUM") as pp2,
        ):
            w1_sb = wpool.tile([P, KC1, H], bf16)
            nc.sync.dma_start(w1_sb[:], w1_d[:].rearrange("(c p) h -> p c h", p=P))
            w2_sb = wpool.tile([P, KC2, H], bf16)
            nc.sync.dma_start(w2_sb[:], w2_d[:].rearrange("(c p) h -> p c h", p=P))
            b1_sb = wpool.tile([1, H], bf16)
            nc.sync.dma_start(b1_sb[:], b1_d[:])
            b2_sb = wpool.tile([1, H], bf16)
            nc.sync.dma_start(b2_sb[:], b2_d[:])
            w3bc = wpool.tile([P, H], bf16)
            nc.gpsimd.dma_start(w3bc[:], w3_d[:].to_broadcast((P, H)))
            b3bc = wpool.tile([P, 1], f32)
            nc.gpsimd.dma_start(b3bc[:], b3_d[:].to_broadcast((P, 1)))
            b3divH = wpool.tile([P, 1], f32)
            nc.vector.tensor_scalar_mul(b3divH[:], b3bc[:], 1.0 / H)

            affine = [None, None]
            if not fast_affine:
                for i, (g_d, be_d) in enumerate(((g1_d, be1_d), (g2_d, be2_d))):
                    gbc = wpool.tile([P, H], bf16, tag=f"g{i}bc")
                    nc.gpsimd.dma_start(gbc[:], g_d[:].to_broadcast((P, H)))
                    bebc = wpool.tile([P, H], bf16, tag=f"be{i}bc")
                    nc.gpsimd.dma_start(bebc[:], be_d[:].to_broadcast((P, H)))
                    affine[i] = (gbc, bebc)

            ones_sb = wpool.tile([1, P], bf16)
            nc.vector.memset(ones_sb[:], 1.0)
            eps_sb = wpool.tile([P, 1], f32)
            nc.vector.memset(eps_sb[:], LN_EPS)
            qstage = wpool.tile([P, NT], f32)

            def ln_relu(h_ps, out_bf, layer_idx):
                """out_bf = relu(layernorm(h_ps) * g + be), h_ps is PSUM fp32."""
                st = stpool.tile([P, 8], f32, tag="st")
                ngrp = H // 512
                bn6 = stpool.tile([P, ngrp, 6], f32, tag="bn6")
                h_grp = h_ps[:].rearrange("p (g f) -> p g f", f=512)
                for g in range(ngrp):
                    nc.vector.bn_stats(bn6[:, g, :], h_grp[:, g, :])
                nc.vector.bn_aggr(st[:, 0:2], bn6[:])  # -> mean, var
                nc.scalar.activation(st[:, 2:3], st[:, 1:2], AF.Sqrt, bias=eps_sb[:])
                nc.vector.reciprocal(st[:, 3:4], st[:, 2:3])  # rs = 1/sqrt(var+eps)
                # nb = -mean * rs
                nc.vector.tensor_scalar(
                    st[:, 4:5], st[:, 0:1], st[:, 3:4], -1.0, OP.mult, OP.mult
                )
                if fast_affine:
                    nc.scalar.activation(
                        out_bf[:], h_ps[:], AF.Relu, bias=st[:, 4:5], scale=st[:, 3:4]
                    )
                else:
                    gbc, bebc = affine[layer_idx]
                    tmp = apool.tile([P, H], bf16, tag="norm_tmp")
                    nc.scalar.activation(
                        tmp[:], h_ps[:], AF.Identity, bias=st[:, 4:5], scale=st[:, 3:4]
                    )
                    tmp2 = apool.tile([P, H], bf16, tag="norm_tmp2")
                    nc.vector.tensor_mul(tmp2[:], tmp[:], gbc[:])
                    nc.vector.tensor_add(tmp2[:], tmp2[:], bebc[:])
                    nc.vector.tensor_scalar_max(out_bf[:], tmp2[:], 0.0)

            for t in range(NT):
                xt = xtpool.tile([P, KC1, P], bf16)
                for c in range(KC1):
                    nc.sync.dma_start_transpose(xt[:, c, :], x_d[ts(t, P), ts(c, P)])

                h1 = pp1.tile([P, H], f32)
                for n in range(H // 512):
                    nsl = ts(n, 512)
                    for c in range(KC1):
                        nc.tensor.matmul(
                            h1[:, nsl],
                            xt[:, c, :],
                            w1_sb[:, c, nsl],
                            start=(c == 0),
                            stop=False,
                        )
                    nc.tensor.matmul(
                        h1[:, nsl], ones_sb[:], b1_sb[:, nsl], start=False, stop=True
                    )

                relu1 = apool.tile([P, H], bf16, tag="relu1")
                ln_relu(h1, relu1, 0)

                rt = rtpool.tile([P, KC2, P], bf16)
                for k in range(KC2):
                    nc.sync.dma_start_transpose(rt[:, k, :], relu1[:, ts(k, P)])

                h2 = pp2.tile([P, H], f32)
                for n in range(H // 512):
                    nsl = ts(n, 512)
                    for k in range(KC2):
                        nc.tensor.matmul(
                            h2[:, nsl],
                            rt[:, k, :],
                            w2_sb[:, k, nsl],
                            start=(k == 0),
                            stop=False,
                        )
                    nc.tensor.matmul(
                        h2[:, nsl], ones_sb[:], b2_sb[:, nsl], start=False, stop=True
                    )

                relu2 = apool.tile([P, H], bf16, tag="relu2")
                ln_relu(h2, relu2, 1)

                # q[:, t] = b3 + sum_h relu2 * W3: DVE elementwise multiply,
                # then ScalarE copy whose accum_out does the row-sum; the
                # per-partition bias b3/H turns into +b3 after accumulation.
                scr = apool.tile([P, H], bf16, tag="l3scr")
                nc.vector.tensor_mul(scr[:], relu2[:], w3bc[:])
                scr2 = apool.tile([P, H], bf16, tag="l3scr2")
                nc.scalar.activation(
                    scr2[:],
                    scr[:],
                    AF.Identity,
                    bias=b3divH[:],
                    accum_out=qstage[:, t : t + 1],
                )

            nc.sync.dma_start(q_d[:], qstage[:])

    _split_transpose_waits(nc, mybir, dummy_sem, replace_range_clear=for_hw)
    _dummy_sem_cm.__exit__(None, None, None)
    return nc


def _split_transpose_waits(nc, mybir, dummy_sem, replace_range_clear=True):
    dummy_sem_id = dummy_sem.num
    """This container's walrus build encodes at most ONE sync-wait command
    per instruction (any more → 'Too many sync wait commands'), and rejects
    the EVENT_SEMAPHORE_RANGE_CLEAR encoding outright ('ISA wrong length').

    Fix both by post-processing the scheduled IR:
    - move excess sync-waits onto InstNoOps inserted just before the
      over-subscribed instruction on the same (in-order) engine stream,
      which preserves ordering semantics exactly;
    - replace the range-clear with per-semaphore sem-sub-imm EventSemaphore
      ops of each semaphore's exact accumulated total (equivalent reset,
      since it runs after the final all-engine barrier).

    Helper instructions are created through the normal bass builders (so
    they serialize with correct lengths) and then relocated."""

    def _fresh_inst(engine_type, builder):
        eng = nc.engines[engine_type]
        inst = builder(eng).ins
        # the builder appended it to the current (end) block; detach it
        for fn in nc.m.functions:
            for blk in fn.blocks:
                il = blk.instructions
                if il and il[-1] is inst:
                    del il[-1]
                    return inst
        raise RuntimeError("could not detach freshly built instruction")

    # per-semaphore totals of all increments in the program
    sem_totals = {}
    for fn in nc.m.functions:
        for blk in fn.blocks:
            for inst in blk.instructions:
                si = inst.sync_info
                if si is None:
                    continue
                for u in si.on_update:
                    if u.sync_type == "semaphore" and u.update_mode == "sem-add-imm":
                        sem_totals[u.id] = sem_totals.get(u.id, 0) + u.update_value

    n_new = 0
    for fn in nc.m.functions:
        for blk in fn.blocks:
            insts = blk.instructions  # live list
            i = 0
            while i < len(insts):
                inst = insts[i]
                nm = type(inst).__name__
                if (
                    replace_range_clear
                    and nm == "InstISA"
                    and getattr(inst, "op_name", "") == "EVENT_SEMAPHORE_RANGE_CLEAR"
                ):
                    eng = inst.engine
                    d = inst.ant_dict
                    first, last = d["range_first"], d["range_last"]
                    del insts[i]
                    for sem_id in range(first, last + 1):
                        tot = sem_totals.get(sem_id, 0)
                        if tot == 0:
                            continue
                        ev = _fresh_inst(eng, lambda e: e.sem_inc(dummy_sem, 1))
                        n_new += 1
                        ev.sync_info = mybir.SyncInfo(
                            on_wait=[],
                            on_update=[
                                mybir.SyncUpdate(
                                    sync_type="semaphore",
                                    id=sem_id,
                                    update_mode="sem-sub-imm",
                                    update_value=tot,
                                )
                            ],
                        )
                        insts.insert(i, ev)
                        i += 1
                    continue
                si = inst.sync_info
                if si is not None and len(si.on_wait) > 1:
                    waits = list(si.on_wait)
                    for w in waits[1:]:
                        nop = _fresh_inst(inst.engine, lambda e: e.nop())
                        n_new += 1
                        nop.sync_info = mybir.SyncInfo(on_wait=[w], on_update=[])
                        insts.insert(i, nop)
                        i += 1
                    inst.sync_info = mybir.SyncInfo(
                        on_wait=waits[:1], on_update=list(si.on_update)
                    )
                i += 1


def _get_program(B, fast_affine):
    key = (B, fast_affine)
    if key not in _PROG_CACHE:
        _PROG_CACHE[key] = _build_program(B, fast_affine)
    return _PROG_CACHE[key]


def kernel(x, W1, b1, g1, be1, W2, b2, g2, be2, W3, b3):
    global _last_results
    x = np.asarray(x, dtype=np.float32)
    W1 = np.asarray(W1, dtype=np.float32)
    b1 = np.asarray(b1, dtype=np.float32)
    g1 = np.asarray(g1, dtype=np.float32)
    be1 = np.asarray(be1, dtype=np.float32)
    W2 = np.asarray(W2, dtype=np.float32)
    b2 = np.asarray(b2, dtype=np.float32)
    g2 = np.asarray(g2, dtype=np.float32)
    be2 = np.asarray(be2, dtype=np.float32)
    W3 = np.asarray(W3, dtype=np.float32)
    b3 = np.asarray(b3, dtype=np.float32)

    B = x.shape[0]
    assert B % P == 0, B
    fast_affine = bool(
        np.all(g1 == 1.0)
        and np.all(be1 == 0.0)
        and np.all(g2 == 1.0)
        and np.all(be2 == 0.0)
    )

    nc = _get_program(B, fast_affine)

    x_bf = x.astype(BF)
    in_maps = []
    for e in range(E):
        m = {
            "x": x_bf,
            "w1": W1[e].astype(BF),
            "w2": W2[e].astype(BF),
            "w3": W3[e].reshape(1, H).astype(BF),
            "b1": b1[e].reshape(1, H).astype(BF),
            "b2": b2[e].reshape(1, H).astype(BF),
            "b3": b3[e].reshape(1, 1).astype(np.float32),
        }
        if not fast_affine:
            m["g1"] = g1[e].reshape(1, H).astype(BF)
            m["be1"] = be1[e].reshape(1, H).astype(BF)
            m["g2"] = g2[e].reshape(1, H).astype(BF)
            m["be2"] = be2[e].reshape(1, H).astype(BF)
        in_maps.append(m)

    from concourse.bass_utils import run_bass_kernel_spmd

    res = run_bass_kernel_spmd(nc, in_maps, core_ids=list(range(E)))
    _last_results = res

    # q[p, t] -> batch row t*P+p; unshard to [E, B, 1] then min over members.
    qs = np.stack(
        [
            np.asarray(res.results[e]["q"], dtype=np.float32).T.reshape(B, 1)
            for e in range(E)
        ]
    )
    q = qs.min(axis=0)
    return (q, qs)


# revision 33
# speedup vs baseline: 1.0669x; 1.0669x over previous
"""Trainium2 Bass kernel: 8-member EnsembleCritic MLP (min-Q over ensemble).

Sharding: expert-parallel — one ensemble member per NeuronCore (E=8 on 8
cores). x is replicated; per-member weights go to their core. Each core
computes qs[e] = MLP_e(x) for its member; the host stacks the per-core
outputs and takes the min over members (the gather step).

Per-core math (batch tiles of 128 rows, batch-major layout [batch=partition,
feature=free]):
  L1:  h1 = x @ W1 + b1           4x K=128 bf16 matmuls + one K=1 bias matmul
  LN1: bn_stats/bn_aggr on PSUM fp32, normalize+ReLU fused in one ScalarE
       activation pass (scale=rsqrt(var+eps), bias=-mu*scale, func=Relu)
  L2:  h2 = relu1 @ W2 + b2       relu1 transposed via DMA-xbar (bf16)
  LN2 + ReLU, then q = sum(relu2 * W3) + b3 via one tensor_tensor_reduce.

Matmul operands are bf16 (full-rate PE, xbar-transposable); all
accumulation/statistics are fp32.
"""

import sys

import numpy as np

if "/opt/trn_rl_repo" not in sys.path:
    sys.path.insert(0, "/opt/trn_rl_repo")

import ml_dtypes

E = 8
D_IN = 512
H = 1024
LN_EPS = 1e-5
P = 128
BF = ml_dtypes.bfloat16

_PROG_CACHE = {}
_last_results = None  # test harness introspection


def _build_program(B, fast_affine, for_hw=True):
    import concourse.bass as bass
    import concourse.tile as tile
    from concourse import mybir
    from concourse.bass import ts

    f32 = mybir.dt.float32
    bf16 = mybir.dt.bfloat16
    AF = mybir.ActivationFunctionType
    OP = mybir.AluOpType

    NT = B // P
    KC1 = D_IN // P  # 4 contraction chunks for layer 1
    KC2 = H // P     # 8 contraction chunks for layer 2

    nc = bass.Bass()
    # real (never-waited) semaphore for the wait-splitting post-pass nops
    _dummy_sem_cm = nc.semaphore("twait_dummy")
    dummy_sem = _dummy_sem_cm.__enter__()
    x_d = nc.dram_tensor("x", [B, D_IN], bf16, kind="ExternalInput")
    w1_d = nc.dram_tensor("w1", [D_IN, H], bf16, kind="ExternalInput")
    w2_d = nc.dram_tensor("w2", [H, H], bf16, kind="ExternalInput")
    w3_d = nc.dram_tensor("w3", [1, H], bf16, kind="ExternalInput")
    b1_d = nc.dram_tensor("b1", [1, H], bf16, kind="ExternalInput")
    b2_d = nc.dram_tensor("b2", [1, H], bf16, kind="ExternalInput")
    b3_d = nc.dram_tensor("b3", [1, 1], f32, kind="ExternalInput")
    if not fast_affine:
        g1_d = nc.dram_tensor("g1", [1, H], bf16, kind="ExternalInput")
        be1_d = nc.dram_tensor("be1", [1, H], bf16, kind="ExternalInput")
        g2_d = nc.dram_tensor("g2", [1, H], bf16, kind="ExternalInput")
        be2_d = nc.dram_tensor("be2", [1, H], bf16, kind="ExternalInput")
    # q[p, t] holds the Q value of batch row t*128+p; host transposes.
    q_d = nc.dram_tensor("q", [P, NT], f32, kind="ExternalOutput")

    with tile.TileContext(nc) as tc:
        with (
            tc.tile_pool(name="weights", bufs=1) as wpool,
            tc.tile_pool(name="xt", bufs=4) as xtpool,
            tc.tile_pool(name="act", bufs=3) as apool,
            tc.tile_pool(name="rt", bufs=3) as rtpool,
            tc.tile_pool(name="stat", bufs=8) as stpool,
            tc.tile_pool(name="psum1", bufs=3, space="PSUM") as pp1,
            tc.tile_pool(name="psum2", bufs=1, space="PSUM") as pp2,
        ):
            w1_sb = wpool.tile([P, KC1, H], bf16)
            nc.sync.dma_start(w1_sb[:], w1_d[:].rearrange("(c p) h -> p c h", p=P))
            w2_sb = wpool.tile([P, KC2, H], bf16)
            nc.sync.dma_start(w2_sb[:], w2_d[:].rearrange("(c p) h -> p c h", p=P))
            b1_sb = wpool.tile([1, H], bf16)
            nc.sync.dma_start(b1_sb[:], b1_d[:])
            b2_sb = wpool.tile([1, H], bf16)
            nc.sync.dma_start(b2_sb[:], b2_d[:])
            w3bc = wpool.tile([P, H], bf16)
            nc.gpsimd.dma_start(w3bc[:], w3_d[:].to_broadcast((P, H)))
            b3bc = wpool.tile([P, 1], f32)
            nc.gpsimd.dma_start(b3bc[:], b3_d[:].to_broadcast((P, 1)))
            b3divH = wpool.tile([P, 1], f32)
            nc.vector.tensor_scalar_mul(b3divH[:], b3bc[:], 1.0 / H)

            affine = [None, None]
            if not fast_affine:
                for i, (g_d, be_d) in enumerate(((g1_d, be1_d), (g2_d, be2_d))):
                    gbc = wpool.tile([P, H], bf16, tag=f"g{i}bc")
                    nc.gpsimd.dma_start(gbc[:], g_d[:].to_broadcast((P, H)))
                    bebc = wpool.tile([P, H], bf16, tag=f"be{i}bc")
                    nc.gpsimd.dma_start(bebc[:], be_d[:].to_broadcast((P, H)))
                    affine[i] = (gbc, bebc)

            ones_sb = wpool.tile([1, P], bf16)
            nc.vector.memset(ones_sb[:], 1.0)
            eps_sb = wpool.tile([P, 1], f32)
            nc.vector.memset(eps_sb[:], LN_EPS)
            qstage = wpool.tile([P, NT], f32)

            def ln_relu(h_ps, out_bf, layer_idx):
                """out_bf = relu(layernorm(h_ps) * g + be), h_ps is PSUM fp32."""
                st = stpool.tile([P, 8], f32, tag="st")
                ngrp = H // 512
                bn6 = stpool.tile([P, ngrp, 6], f32, tag="bn6")
                h_grp = h_ps[:].rearrange("p (g f) -> p g f", f=512)
                for g in range(ngrp):
                    nc.vector.bn_stats(bn6[:, g, :], h_grp[:, g, :])
                nc.vector.bn_aggr(st[:, 0:2], bn6[:])  # -> mean, var
                nc.scalar.activation(st[:, 2:3], st[:, 1:2], AF.Sqrt, bias=eps_sb[:])
                nc.vector.reciprocal(st[:, 3:4], st[:, 2:3])  # rs = 1/sqrt(var+eps)
                # nb = -mean * rs
                nc.vector.tensor_scalar(
                    st[:, 4:5], st[:, 0:1], st[:, 3:4], -1.0, OP.mult, OP.mult
                )
                if fast_affine:
                    nc.scalar.activation(
                        out_bf[:], h_ps[:], AF.Relu, bias=st[:, 4:5], scale=st[:, 3:4]
                    )
                else:
                    gbc, bebc = affine[layer_idx]
                    tmp = apool.tile([P, H], bf16, tag="norm_tmp")
                    nc.scalar.activation(
                        tmp[:], h_ps[:], AF.Identity, bias=st[:, 4:5], scale=st[:, 3:4]
                    )
                    tmp2 = apool.tile([P, H], bf16, tag="norm_tmp2")
                    nc.vector.tensor_mul(tmp2[:], tmp[:], gbc[:])
                    nc.vector.tensor_add(tmp2[:], tmp2[:], bebc[:])
                    nc.vector.tensor_scalar_max(out_bf[:], tmp2[:], 0.0)

            # Software pipeline: layer-1 matmuls run LOOKAHEAD tiles ahead of
            # the rest, so the PE has dense work while tile t's LN chain
            # (DVE/ACT/transpose-DMA) completes. relu1 transposes issue from
            # the ACT engine (program-order after the norm activation), so
            # the SP stream only carries independent x transposes.
            LOOKAHEAD = 2
            h1_of = {}

            def emit_l1(t):
                xt = xtpool.tile([P, KC1, P], bf16, tag="xt")
                for c in range(KC1):
                    nc.sync.dma_start_transpose(xt[:, c, :], x_d[ts(t, P), ts(c, P)])
                h1 = pp1.tile([P, H], f32, tag="h1")
                for n in range(H // 512):
                    nsl = ts(n, 512)
                    for c in range(KC1):
                        nc.tensor.matmul(
                            h1[:, nsl],
                            xt[:, c, :],
                            w1_sb[:, c, nsl],
                            start=(c == 0),
                            stop=False,
                        )
                    nc.tensor.matmul(
                        h1[:, nsl], ones_sb[:], b1_sb[:, nsl], start=False, stop=True
                    )
                h1_of[t] = h1

            def emit_rest(t):
                h1 = h1_of.pop(t)
                relu1 = apool.tile([P, H], bf16, tag="relu1")
                ln_relu(h1, relu1, 0)

                rt = rtpool.tile([P, KC2, P], bf16, tag="rt")
                for k in range(KC2):
                    nc.scalar.dma_start_transpose(rt[:, k, :], relu1[:, ts(k, P)])

                h2 = pp2.tile([P, H], f32, tag="h2")
                for n in range(H // 512):
                    nsl = ts(n, 512)
                    for k in range(KC2):
                        nc.tensor.matmul(
                            h2[:, nsl],
                            rt[:, k, :],
                            w2_sb[:, k, nsl],
                            start=(k == 0),
                            stop=False,
                        )
                    nc.tensor.matmul(
                        h2[:, nsl], ones_sb[:], b2_sb[:, nsl], start=False, stop=True
                    )

                relu2 = apool.tile([P, H], bf16, tag="relu2")
                ln_relu(h2, relu2, 1)

                # q[:, t] = b3 + sum_h relu2 * W3: DVE elementwise multiply,
                # then ScalarE copy whose accum_out does the row-sum; the
                # per-partition bias b3/H turns into +b3 after accumulation.
                scr = apool.tile([P, H], bf16, tag="l3scr")
                nc.vector.tensor_mul(scr[:], relu2[:], w3bc[:])
                scr2 = apool.tile([P, H], bf16, tag="l3scr2")
                nc.scalar.activation(
                    scr2[:],
                    scr[:],
                    AF.Identity,
                    bias=b3divH[:],
                    accum_out=qstage[:, t : t + 1],
                )

            for t in range(min(LOOKAHEAD, NT)):
                emit_l1(t)
            for t in range(NT):
                if t + LOOKAHEAD < NT:
                    emit_l1(t + LOOKAHEAD)
                emit_rest(t)

            nc.sync.dma_start(q_d[:], qstage[:])

    _split_transpose_waits(nc, mybir, dummy_sem, replace_range_clear=for_hw)
    _dummy_sem_cm.__exit__(None, None, None)
    return nc


def _split_transpose_waits(nc, mybir, dummy_sem, replace_range_clear=True):
    dummy_sem_id = dummy_sem.num
    """This container's walrus build encodes at most ONE sync-wait command
    per instruction (any more → 'Too many sync wait commands'), and rejects
    the EVENT_SEMAPHORE_RANGE_CLEAR encoding outright ('ISA wrong length').

    Fix both by post-processing the scheduled IR:
    - move excess sync-waits onto InstNoOps inserted just before the
      over-subscribed instruction on the same (in-order) engine stream,
      which preserves ordering semantics exactly;
    - replace the range-clear with per-semaphore sem-sub-imm EventSemaphore
      ops of each semaphore's exact accumulated total (equivalent reset,
      since it runs after the final all-engine barrier).

    Helper instructions are created through the normal bass builders (so
    they serialize with correct lengths) and then relocated."""

    def _fresh_inst(engine_type, builder):
        eng = nc.engines[engine_type]
        inst = builder(eng).ins
        # the builder appended it to the current (end) block; detach it
        for fn in nc.m.functions:
            for blk in fn.blocks:
                il = blk.instructions
                if il and il[-1] is inst:
                    del il[-1]
                    return inst
        raise RuntimeError("could not detach freshly built instruction")

    # per-semaphore totals of all increments in the program
    sem_totals = {}
    for fn in nc.m.functions:
        for blk in fn.blocks:
            for inst in blk.instructions:
                si = inst.sync_info
                if si is None:
                    continue
                for u in si.on_update:
                    if u.sync_type == "semaphore" and u.update_mode == "sem-add-imm":
                        sem_totals[u.id] = sem_totals.get(u.id, 0) + u.update_value

    n_new = 0
    for fn in nc.m.functions:
        for blk in fn.blocks:
            insts = blk.instructions  # live list
            i = 0
            while i < len(insts):
                inst = insts[i]
                nm = type(inst).__name__
                if (
                    replace_range_clear
                    and nm == "InstISA"
                    and getattr(inst, "op_name", "") == "EVENT_SEMAPHORE_RANGE_CLEAR"
                ):
                    eng = inst.engine
                    d = inst.ant_dict
                    first, last = d["range_first"], d["range_last"]
                    del insts[i]
                    for sem_id in range(first, last + 1):
                        tot = sem_totals.get(sem_id, 0)
                        if tot == 0:
                            continue
                        ev = _fresh_inst(eng, lambda e: e.sem_inc(dummy_sem, 1))
                        n_new += 1
                        ev.sync_info = mybir.SyncInfo(
                            on_wait=[],
                            on_update=[
                                mybir.SyncUpdate(
                                    sync_type="semaphore",
                                    id=sem_id,
                                    update_mode="sem-sub-imm",
                                    update_value=tot,
                                )
                            ],
                        )
                        insts.insert(i, ev)
                        i += 1
                    continue
                si = inst.sync_info
                if si is not None and len(si.on_wait) > 1:
                    waits = list(si.on_wait)
                    for w in waits[1:]:
                        nop = _fresh_inst(inst.engine, lambda e: e.nop())
                        n_new += 1
                        nop.sync_info = mybir.SyncInfo(on_wait=[w], on_update=[])
                        insts.insert(i, nop)
                        i += 1
                    inst.sync_info = mybir.SyncInfo(
                        on_wait=waits[:1], on_update=list(si.on_update)
                    )
                i += 1


def _get_program(B, fast_affine):
    key = (B, fast_affine)
    if key not in _PROG_CACHE:
        _PROG_CACHE[key] = _build_program(B, fast_affine)
    return _PROG_CACHE[key]


def kernel(x, W1, b1, g1, be1, W2, b2, g2, be2, W3, b3):
    global _last_results
    x = np.asarray(x, dtype=np.float32)
    W1 = np.asarray(W1, dtype=np.float32)
    b1 = np.asarray(b1, dtype=np.float32)
    g1 = np.asarray(g1, dtype=np.float32)
    be1 = np.asarray(be1, dtype=np.float32)
    W2 = np.asarray(W2, dtype=np.float32)
    b2 = np.asarray(b2, dtype=np.float32)
    g2 = np.asarray(g2, dtype=np.float32)
    be2 = np.asarray(be2, dtype=np.float32)
    W3 = np.asarray(W3, dtype=np.float32)
    b3 = np.asarray(b3, dtype=np.float32)

    B = x.shape[0]
    assert B % P == 0, B
    fast_affine = bool(
        np.all(g1 == 1.0)
        and np.all(be1 == 0.0)
        and np.all(g2 == 1.0)
        and np.all(be2 == 0.0)
    )

    nc = _get_program(B, fast_affine)

    x_bf = x.astype(BF)
    in_maps = []
    for e in range(E):
        m = {
            "x": x_bf,
            "w1": W1[e].astype(BF),
            "w2": W2[e].astype(BF),
            "w3": W3[e].reshape(1, H).astype(BF),
            "b1": b1[e].reshape(1, H).astype(BF),
            "b2": b2[e].reshape(1, H).astype(BF),
            "b3": b3[e].reshape(1, 1).astype(np.float32),
        }
        if not fast_affine:
            m["g1"] = g1[e].reshape(1, H).astype(BF)
            m["be1"] = be1[e].reshape(1, H).astype(BF)
            m["g2"] = g2[e].reshape(1, H).astype(BF)
            m["be2"] = be2[e].reshape(1, H).astype(BF)
        in_maps.append(m)

    from concourse.bass_utils import run_bass_kernel_spmd

    res = run_bass_kernel_spmd(nc, in_maps, core_ids=list(range(E)))
    _last_results = res

    # q[p, t] -> batch row t*P+p; unshard to [E, B, 1] then min over members.
    qs = np.stack(
        [
            np.asarray(res.results[e]["q"], dtype=np.float32).T.reshape(B, 1)
            for e in range(E)
        ]
    )
    q = qs.min(axis=0)
    return (q, qs)


# revision 37
# speedup vs baseline: 1.7085x; 1.6013x over previous
"""Trainium2 Bass kernel: 8-member EnsembleCritic MLP (min-Q over ensemble).

Sharding: expert-parallel — one ensemble member per NeuronCore (E=8 on 8
cores). x is replicated; per-member weights go to their core. Each core
computes qs[e] = MLP_e(x) for its member; the host stacks the per-core
outputs and takes the min over members (the gather step).

Per-core math (batch tiles of 128 rows, batch-major layout [batch=partition,
feature=free]):
  L1:  h1 = x @ W1 + b1           4x K=128 bf16 matmuls + one K=1 bias matmul
  LN1: bn_stats/bn_aggr on PSUM fp32, normalize+ReLU fused in one ScalarE
       activation pass (scale=rsqrt(var+eps), bias=-mu*scale, func=Relu)
  L2:  h2 = relu1 @ W2 + b2       relu1 transposed via DMA-xbar (bf16)
  LN2 + ReLU, then q = sum(relu2 * W3) + b3 via one tensor_tensor_reduce.

Matmul operands are bf16 (full-rate PE, xbar-transposable); all
accumulation/statistics are fp32.
"""

import sys

import numpy as np

if "/opt/trn_rl_repo" not in sys.path:
    sys.path.insert(0, "/opt/trn_rl_repo")

import ml_dtypes

E = 8
D_IN = 512
H = 1024
LN_EPS = 1e-5
P = 128
BF = ml_dtypes.bfloat16

_PROG_CACHE = {}
_last_results = None  # test harness introspection


def _build_program(B, fast_affine, for_hw=True):
    import concourse.bass as bass
    import concourse.tile as tile
    from concourse import mybir
    from concourse.bass import ts

    f32 = mybir.dt.float32
    bf16 = mybir.dt.bfloat16
    AF = mybir.ActivationFunctionType
    OP = mybir.AluOpType

    NT = B // P
    KC1 = D_IN // P  # 4 contraction chunks for layer 1
    KC2 = H // P     # 8 contraction chunks for layer 2

    nc = bass.Bass()
    # real (never-waited) semaphore for the wait-splitting post-pass nops
    _dummy_sem_cm = nc.semaphore("twait_dummy")
    dummy_sem = _dummy_sem_cm.__enter__()
    x_d = nc.dram_tensor("x", [B, D_IN], bf16, kind="ExternalInput")
    w1_d = nc.dram_tensor("w1", [D_IN, H], bf16, kind="ExternalInput")
    w2_d = nc.dram_tensor("w2", [H, H], bf16, kind="ExternalInput")
    w3_d = nc.dram_tensor("w3", [1, H], bf16, kind="ExternalInput")
    b1_d = nc.dram_tensor("b1", [1, H], bf16, kind="ExternalInput")
    b2_d = nc.dram_tensor("b2", [1, H], bf16, kind="ExternalInput")
    b3_d = nc.dram_tensor("b3", [1, 1], f32, kind="ExternalInput")
    if not fast_affine:
        g1_d = nc.dram_tensor("g1", [1, H], bf16, kind="ExternalInput")
        be1_d = nc.dram_tensor("be1", [1, H], bf16, kind="ExternalInput")
        g2_d = nc.dram_tensor("g2", [1, H], bf16, kind="ExternalInput")
        be2_d = nc.dram_tensor("be2", [1, H], bf16, kind="ExternalInput")
    # q[p, t] holds the Q value of batch row t*128+p; host transposes.
    q_d = nc.dram_tensor("q", [P, NT], f32, kind="ExternalOutput")

    with tile.TileContext(nc) as tc:
        with (
            tc.tile_pool(name="weights", bufs=1) as wpool,
            tc.tile_pool(name="xt", bufs=4) as xtpool,
            tc.tile_pool(name="act", bufs=3) as apool,
            tc.tile_pool(name="rt", bufs=3) as rtpool,
            tc.tile_pool(name="stat", bufs=8) as stpool,
            tc.tile_pool(name="psum1", bufs=3, space="PSUM") as pp1,
            tc.tile_pool(name="psum2", bufs=1, space="PSUM") as pp2,
        ):
            w1_sb = wpool.tile([P, KC1, H], bf16)
            nc.sync.dma_start(w1_sb[:], w1_d[:].rearrange("(c p) h -> p c h", p=P))
            w2_sb = wpool.tile([P, KC2, H], bf16)
            nc.sync.dma_start(w2_sb[:], w2_d[:].rearrange("(c p) h -> p c h", p=P))
            b1_sb = wpool.tile([1, H], bf16)
            nc.sync.dma_start(b1_sb[:], b1_d[:])
            b2_sb = wpool.tile([1, H], bf16)
            nc.sync.dma_start(b2_sb[:], b2_d[:])
            w3bc = wpool.tile([P, H], bf16)
            nc.gpsimd.dma_start(w3bc[:], w3_d[:].to_broadcast((P, H)))
            b3bc = wpool.tile([P, 1], f32)
            nc.gpsimd.dma_start(b3bc[:], b3_d[:].to_broadcast((P, 1)))
            b3divH = wpool.tile([P, 1], f32)
            nc.vector.tensor_scalar_mul(b3divH[:], b3bc[:], 1.0 / H)

            affine = [None, None]
            if not fast_affine:
                for i, (g_d, be_d) in enumerate(((g1_d, be1_d), (g2_d, be2_d))):
                    gbc = wpool.tile([P, H], bf16, tag=f"g{i}bc")
                    nc.gpsimd.dma_start(gbc[:], g_d[:].to_broadcast((P, H)))
                    bebc = wpool.tile([P, H], bf16, tag=f"be{i}bc")
                    nc.gpsimd.dma_start(bebc[:], be_d[:].to_broadcast((P, H)))
                    affine[i] = (gbc, bebc)

            ones_sb = wpool.tile([1, P], bf16)
            nc.vector.memset(ones_sb[:], 1.0)
            eps_sb = wpool.tile([P, 1], f32)
            nc.vector.memset(eps_sb[:], LN_EPS)
            qstage = wpool.tile([P, NT], f32)

            def ln_relu(h_ps, out_bf, layer_idx):
                """out_bf = relu(layernorm(h_ps) * g + be), h_ps is PSUM fp32."""
                st = stpool.tile([P, 8], f32, tag="st")
                ngrp = H // 512
                bn6 = stpool.tile([P, ngrp, 6], f32, tag="bn6")
                h_grp = h_ps[:].rearrange("p (g f) -> p g f", f=512)
                for g in range(ngrp):
                    nc.vector.bn_stats(bn6[:, g, :], h_grp[:, g, :])
                nc.vector.bn_aggr(st[:, 0:2], bn6[:])  # -> mean, var
                nc.scalar.activation(st[:, 2:3], st[:, 1:2], AF.Sqrt, bias=eps_sb[:])
                nc.vector.reciprocal(st[:, 3:4], st[:, 2:3])  # rs = 1/sqrt(var+eps)
                # nb = -mean * rs
                nc.vector.tensor_scalar(
                    st[:, 4:5], st[:, 0:1], st[:, 3:4], -1.0, OP.mult, OP.mult
                )
                if fast_affine:
                    nc.scalar.activation(
                        out_bf[:], h_ps[:], AF.Relu, bias=st[:, 4:5], scale=st[:, 3:4]
                    )
                else:
                    gbc, bebc = affine[layer_idx]
                    tmp = apool.tile([P, H], bf16, tag="norm_tmp")
                    nc.scalar.activation(
                        tmp[:], h_ps[:], AF.Identity, bias=st[:, 4:5], scale=st[:, 3:4]
                    )
                    tmp2 = apool.tile([P, H], bf16, tag="norm_tmp2")
                    nc.vector.tensor_mul(tmp2[:], tmp[:], gbc[:])
                    nc.vector.tensor_add(tmp2[:], tmp2[:], bebc[:])
                    nc.vector.tensor_scalar_max(out_bf[:], tmp2[:], 0.0)

            # Software pipeline: layer-1 matmuls run LOOKAHEAD tiles ahead of
            # the rest, so the PE has dense work while tile t's LN chain
            # (DVE/ACT/transpose-DMA) completes. relu1 transposes issue from
            # the ACT engine (program-order after the norm activation), so
            # the SP stream only carries independent x transposes.
            LOOKAHEAD = 2
            h1_of = {}

            def emit_l1(t):
                xt = xtpool.tile([P, KC1, P], bf16, tag="xt")
                nc.sync.dma_start_transpose(xt[:], x_d[ts(t, P), :])
                h1 = pp1.tile([P, H], f32, tag="h1")
                for n in range(H // 512):
                    nsl = ts(n, 512)
                    for c in range(KC1):
                        nc.tensor.matmul(
                            h1[:, nsl],
                            xt[:, c, :],
                            w1_sb[:, c, nsl],
                            start=(c == 0),
                            stop=False,
                        )
                    nc.tensor.matmul(
                        h1[:, nsl], ones_sb[:], b1_sb[:, nsl], start=False, stop=True
                    )
                h1_of[t] = h1

            def emit_rest(t):
                h1 = h1_of.pop(t)
                relu1 = apool.tile([P, H], bf16, tag="relu1")
                ln_relu(h1, relu1, 0)

                rt = rtpool.tile([P, KC2, P], bf16, tag="rt")
                nc.scalar.dma_start_transpose(rt[:, 0:4, :], relu1[:, 0:512])
                nc.scalar.dma_start_transpose(rt[:, 4:8, :], relu1[:, 512:1024])

                h2 = pp2.tile([P, H], f32, tag="h2")
                for n in range(H // 512):
                    nsl = ts(n, 512)
                    for k in range(KC2):
                        nc.tensor.matmul(
                            h2[:, nsl],
                            rt[:, k, :],
                            w2_sb[:, k, nsl],
                            start=(k == 0),
                            stop=False,
                        )
                    nc.tensor.matmul(
                        h2[:, nsl], ones_sb[:], b2_sb[:, nsl], start=False, stop=True
                    )

                relu2 = apool.tile([P, H], bf16, tag="relu2")
                ln_relu(h2, relu2, 1)

                # q[:, t] = b3 + sum_h relu2 * W3: DVE elementwise multiply,
                # then ScalarE copy whose accum_out does the row-sum; the
                # per-partition bias b3/H turns into +b3 after accumulation.
                scr = apool.tile([P, H], bf16, tag="l3scr")
                nc.vector.tensor_mul(scr[:], relu2[:], w3bc[:])
                scr2 = apool.tile([P, H], bf16, tag="l3scr2")
                nc.scalar.activation(
                    scr2[:],
                    scr[:],
                    AF.Identity,
                    bias=b3divH[:],
                    accum_out=qstage[:, t : t + 1],
                )

            # all startup copy-DMAs (weights, broadcasts) must complete before
            # the first xbar transposes: the xbar-mode transition tracking
            # only serializes against the most recent copy lane, and a
            # still-draining weight load corrupts early transposes.
            tc.strict_bb_all_engine_barrier()

            for t in range(min(LOOKAHEAD, NT)):
                emit_l1(t)
            for t in range(NT):
                if t + LOOKAHEAD < NT:
                    emit_l1(t + LOOKAHEAD)
                emit_rest(t)

            nc.sync.dma_start(q_d[:], qstage[:])

    _split_transpose_waits(nc, mybir, dummy_sem, replace_range_clear=for_hw)
    _dummy_sem_cm.__exit__(None, None, None)
    return nc


def _split_transpose_waits(nc, mybir, dummy_sem, replace_range_clear=True):
    dummy_sem_id = dummy_sem.num
    """This container's walrus build encodes at most ONE sync-wait command
    per instruction (any more → 'Too many sync wait commands'), and rejects
    the EVENT_SEMAPHORE_RANGE_CLEAR encoding outright ('ISA wrong length').

    Fix both by post-processing the scheduled IR:
    - move excess sync-waits onto InstNoOps inserted just before the
      over-subscribed instruction on the same (in-order) engine stream,
      which preserves ordering semantics exactly;
    - replace the range-clear with per-semaphore sem-sub-imm EventSemaphore
      ops of each semaphore's exact accumulated total (equivalent reset,
      since it runs after the final all-engine barrier).

    Helper instructions are created through the normal bass builders (so
    they serialize with correct lengths) and then relocated."""

    def _fresh_inst(engine_type, builder):
        eng = nc.engines[engine_type]
        inst = builder(eng).ins
        # the builder appended it to the current (end) block; detach it
        for fn in nc.m.functions:
            for blk in fn.blocks:
                il = blk.instructions
                if il and il[-1] is inst:
                    del il[-1]
                    return inst
        raise RuntimeError("could not detach freshly built instruction")

    # per-semaphore totals of all increments in the program
    sem_totals = {}
    for fn in nc.m.functions:
        for blk in fn.blocks:
            for inst in blk.instructions:
                si = inst.sync_info
                if si is None:
                    continue
                for u in si.on_update:
                    if u.sync_type == "semaphore" and u.update_mode == "sem-add-imm":
                        sem_totals[u.id] = sem_totals.get(u.id, 0) + u.update_value

    n_new = 0
    for fn in nc.m.functions:
        for blk in fn.blocks:
            insts = blk.instructions  # live list
            i = 0
            while i < len(insts):
                inst = insts[i]
                nm = type(inst).__name__
                if (
                    replace_range_clear
                    and nm == "InstISA"
                    and getattr(inst, "op_name", "") == "EVENT_SEMAPHORE_RANGE_CLEAR"
                ):
                    eng = inst.engine
                    d = inst.ant_dict
                    first, last = d["range_first"], d["range_last"]
                    del insts[i]
                    for sem_id in range(first, last + 1):
                        tot = sem_totals.get(sem_id, 0)
                        if tot == 0:
                            continue
                        ev = _fresh_inst(eng, lambda e: e.sem_inc(dummy_sem, 1))
                        n_new += 1
                        ev.sync_info = mybir.SyncInfo(
                            on_wait=[],
                            on_update=[
                                mybir.SyncUpdate(
                                    sync_type="semaphore",
                                    id=sem_id,
                                    update_mode="sem-sub-imm",
                                    update_value=tot,
                                )
                            ],
                        )
                        insts.insert(i, ev)
                        i += 1
                    continue
                si = inst.sync_info
                if si is not None and len(si.on_wait) > 1:
                    waits = list(si.on_wait)
                    for w in waits[1:]:
                        nop = _fresh_inst(inst.engine, lambda e: e.nop())
                        n_new += 1
                        nop.sync_info = mybir.SyncInfo(on_wait=[w], on_update=[])
                        insts.insert(i, nop)
                        i += 1
                    inst.sync_info = mybir.SyncInfo(
                        on_wait=waits[:1], on_update=list(si.on_update)
                    )
                i += 1


def _get_program(B, fast_affine):
    key = (B, fast_affine)
    if key not in _PROG_CACHE:
        _PROG_CACHE[key] = _build_program(B, fast_affine)
    return _PROG_CACHE[key]


def kernel(x, W1, b1, g1, be1, W2, b2, g2, be2, W3, b3):
    global _last_results
    x = np.asarray(x, dtype=np.float32)
    W1 = np.asarray(W1, dtype=np.float32)
    b1 = np.asarray(b1, dtype=np.float32)
    g1 = np.asarray(g1, dtype=np.float32)
    be1 = np.asarray(be1, dtype=np.float32)
    W2 = np.asarray(W2, dtype=np.float32)
    b2 = np.asarray(b2, dtype=np.float32)
    g2 = np.asarray(g2, dtype=np.float32)
    be2 = np.asarray(be2, dtype=np.float32)
    W3 = np.asarray(W3, dtype=np.float32)
    b3 = np.asarray(b3, dtype=np.float32)

    B = x.shape[0]
    assert B % P == 0, B
    fast_affine = bool(
        np.all(g1 == 1.0)
        and np.all(be1 == 0.0)
        and np.all(g2 == 1.0)
        and np.all(be2 == 0.0)
    )

    nc = _get_program(B, fast_affine)

    x_bf = x.astype(BF)
    in_maps = []
    for e in range(E):
        m = {
            "x": x_bf,
            "w1": W1[e].astype(BF),
            "w2": W2[e].astype(BF),
            "w3": W3[e].reshape(1, H).astype(BF),
            "b1": b1[e].reshape(1, H).astype(BF),
            "b2": b2[e].reshape(1, H).astype(BF),
            "b3": b3[e].reshape(1, 1).astype(np.float32),
        }
        if not fast_affine:
            m["g1"] = g1[e].reshape(1, H).astype(BF)
            m["be1"] = be1[e].reshape(1, H).astype(BF)
            m["g2"] = g2[e].reshape(1, H).astype(BF)
            m["be2"] = be2[e].reshape(1, H).astype(BF)
        in_maps.append(m)

    from concourse.bass_utils import run_bass_kernel_spmd

    res = run_bass_kernel_spmd(nc, in_maps, core_ids=list(range(E)))
    _last_results = res

    # q[p, t] -> batch row t*P+p; unshard to [E, B, 1] then min over members.
    qs = np.stack(
        [
            np.asarray(res.results[e]["q"], dtype=np.float32).T.reshape(B, 1)
            for e in range(E)
        ]
    )
    q = qs.min(axis=0)
    return (q, qs)
